# revision 10
# baseline (speedup 1.0000x reference)
"""KmeansVectorQuantizer forward on 8 Trainium2 NeuronCores.

Data-parallel over batch (2 batches/core).  Per core, per group:
  grouped 1x1 conv (PE fp32 matmul) -> GroupNorm over (T, D) -> VQ:
  neg_d2 = 2*<ze,e> - |ze|^2 - |e|^2 (PE matmul + DVE), argmin via
  InstMax/InstMaxIndex (lowest-index tie-break, matching jnp.argmin),
  codebook gather via indirect DMA, straight-through x_out.
Scalar outputs (kmeans loss, perplexity) are reassembled on host from
per-position max values (max(neg_d2) == -|zq-ze|^2) and the index
histogram; the hot path never materializes them on device.
"""

import numpy as np

import concourse.bass as bass
import concourse.bacc as bacc
import concourse.tile as tile
import concourse.mybir as mybir
from concourse import bass_isa
from concourse.bass_utils import run_bass_kernel_spmd
from concourse.masks import make_identity

B, T, C = 16, 2048, 512
G, D, V = 2, 256, 320
NCORES = 8
BL = B // NCORES            # local batches per core
NL = BL * T                 # local positions per core
NCHUNK = NL // 128          # 128-position chunks per group
GAMMA = np.float32(0.25)
EPS_GN = np.float32(1e-5)
EPS_PPL = np.float32(1e-7)

_programs = {}

f32 = mybir.dt.float32
u32 = mybir.dt.uint32


def _build(apply_gn: bool, num_devices: int = NCORES):
    nc = bacc.Bacc("TRN2", target_bir_lowering=False, debug=False,
                   num_devices=num_devices)

    d_xt = nc.dram_tensor("xt", [G, 2, 128, NL], f32, kind="ExternalInput").ap()
    d_wt = nc.dram_tensor("wt", [G, 2, 128, D], f32, kind="ExternalInput").ap()
    d_e2t = nc.dram_tensor("e2t", [G, 2, 128, V], f32, kind="ExternalInput").ap()
    d_esq = nc.dram_tensor("esq", [128, G, V], f32, kind="ExternalInput").ap()
    d_etab = [nc.dram_tensor(f"etab{g}", [V, D], f32, kind="ExternalInput").ap()
              for g in range(G)]
    d_gnw = nc.dram_tensor("gnw", [G, 2, 128, 1], f32, kind="ExternalInput").ap()
    d_gnb = nc.dram_tensor("gnb", [G, 2, 128, 1], f32, kind="ExternalInput").ap()
    d_xout = nc.dram_tensor("xout", [NL, C], f32, kind="ExternalOutput").ap()
    d_idx8 = nc.dram_tensor("idx8", [G, NCHUNK, 128, 8], u32, kind="ExternalOutput").ap()
    d_max8 = nc.dram_tensor("max8", [G, NCHUNK, 128, 8], f32, kind="ExternalOutput").ap()

    from contextlib import ExitStack
    with tile.TileContext(nc) as tc, ExitStack() as ctx:
        consts = ctx.enter_context(tc.tile_pool(name="consts", bufs=1))
        ypool = ctx.enter_context(tc.tile_pool(name="y", bufs=1))
        xpool = ctx.enter_context(tc.tile_pool(name="xt", bufs=1))
        psA = ctx.enter_context(tc.tile_pool(name="psA", bufs=3, space="PSUM"))
        stat = ctx.enter_context(tc.tile_pool(name="stat", bufs=1))
        pst = ctx.enter_context(tc.tile_pool(name="pst", bufs=2, space="PSUM"))
        psd = ctx.enter_context(tc.tile_pool(name="psd", bufs=3, space="PSUM"))
        work = ctx.enter_context(tc.tile_pool(name="work", bufs=4))
        small = ctx.enter_context(tc.tile_pool(name="small", bufs=8))

        ident = consts.tile([128, 128], f32, tag="ident")
        make_identity(nc, ident[:])

        wt_t = [[consts.tile([128, D], f32, tag=f"wt{g}{ic}", name=f"wt{g}{ic}") for ic in range(2)]
                for g in range(G)]
        e2_t = [[consts.tile([128, V], f32, tag=f"e2{g}{ic}", name=f"e2{g}{ic}") for ic in range(2)]
                for g in range(G)]
        esq_t = consts.tile([128, G, V], f32, tag="esq")
        nc.sync.dma_start(esq_t[:], d_esq)
        for g in range(G):
            for ic in range(2):
                nc.sync.dma_start(wt_t[g][ic][:], d_wt[g, ic])
                nc.sync.dma_start(e2_t[g][ic][:], d_e2t[g, ic])
        if apply_gn:
            gnw_t = [[consts.tile([128, 1], f32, tag=f"gw{g}{dc}", name=f"gw{g}{dc}") for dc in range(2)]
                     for g in range(G)]
            gnb_t = [[consts.tile([128, 1], f32, tag=f"gb{g}{dc}", name=f"gb{g}{dc}") for dc in range(2)]
                     for g in range(G)]
            for g in range(G):
                for dc in range(2):
                    nc.sync.dma_start(gnw_t[g][dc][:], d_gnw[g, dc])
                    nc.sync.dma_start(gnb_t[g][dc][:], d_gnb[g, dc])

        xt_t = [[xpool.tile([128, NL], f32, tag=f"x{g}{ic}", name=f"x{g}{ic}") for ic in range(2)]
                for g in range(G)]
        for g in range(G):
            for ic in range(2):
                for j in range(NL // 512):
                    nc.sync.dma_start(xt_t[g][ic][:, j * 512:(j + 1) * 512],
                                      d_xt[g, ic, :, j * 512:(j + 1) * 512])

        y_t = [[ypool.tile([128, NL], f32, tag=f"y{g}{dc}", name=f"y{g}{dc}") for dc in range(2)]
               for g in range(G)]

        # ---- Phase A: grouped 1x1 conv, y = w.T @ x ----
        for g in range(G):
            for dc in range(2):
                for j in range(NL // 512):
                    ps = psA.tile([128, 512], f32, tag="psA")
                    nc.tensor.matmul(ps[:], wt_t[g][0][:, dc * 128:(dc + 1) * 128],
                                     xt_t[g][0][:, j * 512:(j + 1) * 512],
                                     start=True, stop=False)
                    nc.tensor.matmul(ps[:], wt_t[g][1][:, dc * 128:(dc + 1) * 128],
                                     xt_t[g][1][:, j * 512:(j + 1) * 512],
                                     start=False, stop=True)
                    nc.scalar.copy(y_t[g][dc][:, j * 512:(j + 1) * 512], ps[:])

        # ---- Phase A2: GroupNorm stats over (T, D) per (batch, group) ----
        mu_t = [[None] * BL for _ in range(G)]
        rstd_t = [[None] * BL for _ in range(G)]
        for g in range(G):
            for b in range(BL):
                s3 = [None, None]
                for dc in range(2):
                    st6 = stat.tile([128, 4, 6], f32, tag="st6")
                    for a in range(4):
                        nc.vector.bn_stats(
                            st6[:, a, :],
                            y_t[g][dc][:, b * T + a * 512: b * T + (a + 1) * 512])
                    mv = stat.tile([128, 2], f32, tag="mv")
                    nc.vector.bn_aggr(mv[:], st6[:])
                    pk = stat.tile([128, 3], f32, tag="pk")
                    nc.vector.tensor_copy(pk[:, 0:2], mv[:])
                    nc.vector.tensor_mul(pk[:, 2:3], mv[:, 0:1], mv[:, 0:1])
                    s3[dc] = stat.tile([128, 3], f32, tag=f"s3{dc}", name=f"s3{dc}")
                    nc.gpsimd.partition_all_reduce(
                        s3[dc][:], pk[:], channels=128,
                        reduce_op=bass_isa.ReduceOp.add)
                tot = stat.tile([128, 3], f32, tag="tot")
                nc.vector.tensor_add(tot[:], s3[0][:], s3[1][:])
                mu = stat.tile([128, 1], f32, tag=f"mu{g}{b}")
                nc.vector.tensor_scalar_mul(mu[:], tot[:, 0:1], 1.0 / 256.0)
                ev = stat.tile([128, 1], f32, tag="ev")
                nc.vector.tensor_add(ev[:], tot[:, 1:2], tot[:, 2:3])
                nc.vector.tensor_scalar_mul(ev[:], ev[:], 1.0 / 256.0)
                mu2 = stat.tile([128, 1], f32, tag="mu2")
                nc.vector.tensor_mul(mu2[:], mu[:], mu[:])
                veps = stat.tile([128, 1], f32, tag="veps")
                nc.vector.tensor_sub(veps[:], ev[:], mu2[:])
                nc.vector.tensor_scalar_add(veps[:], veps[:], float(EPS_GN))
                s0 = stat.tile([128, 1], f32, tag="s0")
                nc.scalar.sqrt(s0[:], veps[:])
                r = stat.tile([128, 1], f32, tag=f"r{g}{b}")
                nc.vector.reciprocal(r[:], s0[:])
                t1 = stat.tile([128, 1], f32, tag="t1")
                for _ in range(2):  # Newton: r <- r*(1.5 - 0.5*v*r*r)
                    nc.vector.tensor_mul(t1[:], r[:], r[:])
                    nc.vector.tensor_mul(t1[:], t1[:], veps[:])
                    nc.vector.tensor_scalar(t1[:], t1[:], -0.5, 1.5,
                                            op0=mybir.AluOpType.mult,
                                            op1=mybir.AluOpType.add)
                    nc.vector.tensor_mul(r[:], r[:], t1[:])
                mu_t[g][b] = mu
                rstd_t[g][b] = r

        # ---- Phase B: normalize in place, y -> ze ----
        for g in range(G):
            for dc in range(2):
                for b in range(BL):
                    for a in range(4):
                        zev = y_t[g][dc][:, b * T + a * 512: b * T + (a + 1) * 512]
                        nc.vector.tensor_scalar(zev, zev, mu_t[g][b][:],
                                                rstd_t[g][b][:],
                                                op0=mybir.AluOpType.subtract,
                                                op1=mybir.AluOpType.mult)
                if apply_gn:
                    zev = y_t[g][dc][:]
                    nc.vector.tensor_scalar(zev, zev, gnw_t[g][dc][:],
                                            gnb_t[g][dc][:],
                                            op0=mybir.AluOpType.mult,
                                            op1=mybir.AluOpType.add)

        # ---- Phase C: VQ per 128-position chunk ----
        for g in range(G):
            for c in range(NCHUNK):
                cs = slice(c * 128, (c + 1) * 128)
                # transpose ze chunk -> [n, d]
                pt = pst.tile([128, 256], f32, tag="pt")
                for dc in range(2):
                    nc.tensor.transpose(pt[:, dc * 128:(dc + 1) * 128],
                                        y_t[g][dc][:, cs], ident[:])
                zeT = work.tile([128, 256], f32, tag="zeT")
                nc.scalar.copy(zeT[:], pt[:])
                # dots2 = 2*ze.e
                pd = psd.tile([128, V], f32, tag="pd")
                nc.tensor.matmul(pd[:], y_t[g][0][:, cs], e2_t[g][0][:],
                                 start=True, stop=False)
                nc.tensor.matmul(pd[:], y_t[g][1][:, cs], e2_t[g][1][:],
                                 start=False, stop=True)
                # z_sq = sum(ze^2) via ACT Square with per-partition accumulate
                sq = work.tile([128, 256], f32, tag="sq")
                zsq = small.tile([128, 1], f32, tag="zsq")
                nc.scalar.activation(sq[:], zeT[:],
                                     mybir.ActivationFunctionType.Square,
                                     accum_out=zsq[:])
                # neg_d2 = (dots2 - z_sq) - e_sq
                ndt = work.tile([128, V], f32, tag="ndt")
                nc.vector.scalar_tensor_tensor(
                    ndt[:], pd[:], zsq[:], esq_t[:, g, :],
                    op0=mybir.AluOpType.subtract,
                    op1=mybir.AluOpType.subtract)
                mx8 = small.tile([128, 8], f32, tag="mx8")
                ix8 = small.tile([128, 8], u32, tag="ix8")
                nc.vector.max(mx8[:], ndt[:])
                nc.vector.max_index(ix8[:], mx8[:], ndt[:])
                nc.sync.dma_start(d_max8[g, c], mx8[:])
                nc.sync.dma_start(d_idx8[g, c], ix8[:])
                # gather codebook rows
                zq = work.tile([128, 256], f32, tag="zq")
                nc.gpsimd.indirect_dma_start(
                    out=zq[:], out_offset=None, in_=d_etab[g],
                    in_offset=bass.IndirectOffsetOnAxis(ap=ix8[:, 0:1], axis=0))
                # x_out = (zq + ze) - ze
                s = work.tile([128, 256], f32, tag="s")
                nc.vector.tensor_add(s[:], zq[:], zeT[:])
                xo = work.tile([128, 256], f32, tag="xo")
                nc.gpsimd.tensor_sub(xo[:], s[:], zeT[:])
                nc.sync.dma_start(d_xout[cs, g * 256:(g + 1) * 256], xo[:])

    nc.compile()
    return nc


def _get_program(apply_gn: bool):
    if apply_gn not in _programs:
        _programs[apply_gn] = _build(apply_gn)
    return _programs[apply_gn]


def kernel(x, conv_w, gn_w, gn_b, embedding, _profile=None):
    x = np.ascontiguousarray(np.asarray(x, dtype=np.float32))
    conv_w = np.asarray(conv_w, dtype=np.float32)
    gn_w = np.asarray(gn_w, dtype=np.float32)
    gn_b = np.asarray(gn_b, dtype=np.float32)
    embedding = np.asarray(embedding, dtype=np.float32)

    apply_gn = not (np.all(gn_w == 1.0) and np.all(gn_b == 0.0))
    nc = _get_program(apply_gn)
    in_maps = _prep_inputs(x, conv_w, gn_w, gn_b, embedding)

    kw = dict(_profile) if _profile else {}
    out = run_bass_kernel_spmd(nc, in_maps, core_ids=list(range(NCORES)), **kw)
    res = out.results
    return _postprocess(res, out, _profile)


def _prep_inputs(x, conv_w, gn_w, gn_b, embedding):
    # host-side input marshaling
    wt = np.ascontiguousarray(
        conv_w.transpose(0, 2, 1).reshape(G, 2, 128, D))
    e2t = np.ascontiguousarray(
        (np.float32(2.0) * embedding).transpose(1, 2, 0).reshape(G, 2, 128, V))
    e_sq = np.einsum("vgd,vgd->gv", embedding, embedding, dtype=np.float32
                     ).astype(np.float32)
    esq = np.ascontiguousarray(np.broadcast_to(e_sq[None], (128, G, V)))
    etab = [np.ascontiguousarray(embedding[:, g, :]) for g in range(G)]
    gnw_s = np.ascontiguousarray(gn_w.reshape(G, 2, 128, 1))
    gnb_s = np.ascontiguousarray(gn_b.reshape(G, 2, 128, 1))

    xs = x.reshape(NCORES, NL, G, 2, 128)
    in_maps = []
    for core in range(NCORES):
        xt = np.ascontiguousarray(xs[core].transpose(1, 2, 3, 0))
        in_maps.append({
            "xt": xt, "wt": wt, "e2t": e2t, "esq": esq,
            "etab0": etab[0], "etab1": etab[1],
            "gnw": gnw_s, "gnb": gnb_s,
        })

    return in_maps


def _postprocess(res, out=None, _profile=None):
    x_out = np.concatenate([r["xout"].reshape(BL, T, C) for r in res], axis=0)
    idx8 = np.stack([r["idx8"] for r in res])        # [8, G, NCHUNK, 128, 8]
    max8 = np.stack([r["max8"] for r in res])
    idx = idx8[..., 0].astype(np.int32)              # [8, G, NCHUNK, 128]
    idx_full = idx.transpose(0, 2, 3, 1).reshape(B, T, G)
    maxv = max8[..., 0].astype(np.float64)

    # kmeans loss: sum over positions of |zq-ze|^2 == -sum(maxv)
    m = np.float32(-maxv.sum() / float(B * T * C))
    kmeans_loss = np.float32(m + GAMMA * m)

    # perplexity from the index histogram, fp32 chain mirroring reference
    cnt = np.zeros((G, V), np.int64)
    for g in range(G):
        cnt[g] = np.bincount(idx_full[:, :, g].ravel(), minlength=V)
    probs = (cnt / np.float64(B * T)).astype(np.float32)
    lg = np.log(probs + EPS_PPL).astype(np.float32)
    ent = -(probs * lg).astype(np.float32).sum(axis=-1, dtype=np.float32)
    ppl = np.exp(ent.astype(np.float32)).astype(np.float32).sum(dtype=np.float32)

    if _profile is not None and isinstance(_profile, dict):
        _profile["_result"] = out
    return x_out, idx_full, ppl, kmeans_loss
